# revision 1
# baseline (speedup 1.0000x reference)
"""MoE layer (N=4096, D=1024, H=4096, E=8, top-2) on 8 Trainium2 cores.

Strategy (expert-parallel, per the sharding hint):
  - Host computes the tiny gate (x @ Wg + bg), top-2 expert ids and softmax
    weights, then dispatches each token's row to its experts' cores
    (the host-side shard step IS the all-to-all dispatch).
  - Core e holds expert e's weights and runs the FFN for the <=C tokens
    routed to it:  y_e = relu(x_e @ W1[e] + b1[e]) @ W2[e].
  - Host combines: out[tok] += w_tok * (y_e[tok] + b2[e])  (scatter-add).

Device kernel (identical SPMD program on all 8 cores):
  - All matmuls run as float32r (~tf32 precision, bf16-rate on the PE).
  - Activations stay "transposed" (hT: hidden on partitions, tokens on the
    free axis) so both gemms consume natural weight layouts:
      gemm1: hT[h,t] += W1[dk,h].T @ xT[dk,t]   (stationary W1 tile)
      gemm2: y[t,d]  += hT[hk,t].T @ W2[hk,d]   (stationary hT tile)
  - H is processed in slabs: W1 streams in 512-wide chunks (small first
    chunk -> PE starts early), W2 in 1024-wide slabs so y accumulates in
    SBUF with only 4 add passes. All weight pools single-buffered; loads
    hide under the opposite gemm of the pipeline.
"""

import numpy as np

from concourse import bacc
import concourse.mybir as mybir
from concourse.tile import TileContext
import concourse.bass_utils as bass_utils

N_TOK, D, H, E, TOPK = 4096, 1024, 4096, 8, 2
NCORES = 8
C = 1120  # per-expert token capacity (max observed count 1091; last token tile is partial)
TOK_SLICES = [(0, 384), (384, 384), (768, 352)]  # all >=256 for fp32r rate
SLAB1 = 512  # gemm1 (W1) hidden chunk
SLAB2 = 1024  # gemm2 (W2) hidden slab; y adds once per slab
assert sum(t[1] for t in TOK_SLICES) == C

TRACE = False
TRACE_CORES = None
LAST_RESULTS = None

_NC_CACHE = {}


def _build_nc():
    f32, f32r = mybir.dt.float32, mybir.dt.float32r
    nc = bacc.Bacc("TRN2", target_bir_lowering=False)
    xT = nc.dram_tensor("xT", [D, C], f32r, kind="ExternalInput")
    W1 = nc.dram_tensor("W1", [D, H], f32r, kind="ExternalInput")
    W2 = nc.dram_tensor("W2", [H, D], f32r, kind="ExternalInput")
    b1 = nc.dram_tensor("b1", [H, 1], f32, kind="ExternalInput")
    y = nc.dram_tensor("y", [C, D], f32, kind="ExternalOutput")

    n_dk = D // 128  # 8
    n_s2 = H // SLAB2  # 4 gemm2 slabs
    n_half = SLAB2 // SLAB1  # 2 gemm1 chunks per gemm2 slab
    n_hm = SLAB1 // 128  # 4
    n_hk = SLAB2 // 128  # 8
    n_tk = (C + 127) // 128  # 9 (last tile partial: 96 tokens)
    n_dn = D // 512  # 2
    add, mx = mybir.AluOpType.add, mybir.AluOpType.max

    with TileContext(nc) as tc:
        with (
            tc.tile_pool(name="xp", bufs=1) as xp,
            tc.tile_pool(name="w1p", bufs=2) as w1p,
            tc.tile_pool(name="w2p", bufs=1) as w2p,
            tc.tile_pool(name="hp", bufs=1) as hp,
            tc.tile_pool(name="yp", bufs=1) as yp,
            tc.tile_pool(name="cp", bufs=2) as cp,
            tc.tile_pool(name="ps1", bufs=4, space="PSUM") as ps1,
            tc.tile_pool(name="ps2", bufs=4, space="PSUM") as ps2,
        ):
            _dma_i = [0]

            def hwdma(**kw):
                eng = (nc.sync, nc.scalar)[_dma_i[0] % 2]
                _dma_i[0] += 1
                eng.dma_start(**kw)

            # --- HAM warmup: dummy matmuls on a zeroed tile run during the
            # initial weight/activation DMA wait so the PE clock-gate is
            # already released (2.4 GHz) when real work arrives ---
            warm = xp.tile([128, 512], mybir.dt.bfloat16, name="warm")
            nc.vector.memset(warm, 0.0)
            wps = ps1.tile([128, 384], f32, tag="ps1", name="warmps")
            for i in range(44):
                nc.tensor.matmul(
                    wps, warm[:, :128], warm[:, :384], start=(i == 0), stop=(i == 43)
                )

            # --- startup: first W1 chunk + xT token-slice 0 first ---
            w1t = []
            for dk in range(n_dk):
                t = w1p.tile([128, SLAB1], f32r, tag=f"w1_{dk}", name=f"w1t{dk}")
                hwdma(out=t, in_=W1[dk * 128 : (dk + 1) * 128, 0:SLAB1])
                w1t.append(t)
            xt = []
            for dk in range(n_dk):
                t = xp.tile([128, C], f32r, tag=f"x{dk}", name=f"xt{dk}")
                t0, tn = TOK_SLICES[0]
                hwdma(
                    out=t[:, t0 : t0 + tn],
                    in_=xT[dk * 128 : (dk + 1) * 128, t0 : t0 + tn],
                )
                xt.append(t)
            for t0, tn in TOK_SLICES[1:]:
                for dk in range(n_dk):
                    hwdma(
                        out=xt[dk][:, t0 : t0 + tn],
                        in_=xT[dk * 128 : (dk + 1) * 128, t0 : t0 + tn],
                    )

            yt = [
                yp.tile([128, D], f32, tag=f"y{tk}", name=f"y{tk}")
                for tk in range(n_tk)
            ]

            for s2 in range(n_s2):
                hts = []
                for half in range(n_half):
                    s1 = s2 * n_half + half
                    if s1 > 0:  # chunk 0 loaded in the preamble
                        w1t = []
                        for dk in range(n_dk):
                            t = w1p.tile(
                                [128, SLAB1], f32r, tag=f"w1_{dk}", name=f"w1t{dk}"
                            )
                            h0 = s1 * SLAB1
                            hwdma(
                                out=t, in_=W1[dk * 128 : (dk + 1) * 128, h0 : h0 + SLAB1]
                            )
                            w1t.append(t)
                    b1t = []
                    for hm in range(n_hm):
                        t = cp.tile([128, 1], f32, tag=f"b1_{hm}", name=f"b1t{hm}")
                        h0 = s1 * SLAB1 + hm * 128
                        nc.gpsimd.dma_start(out=t, in_=b1[h0 : h0 + 128, :])
                        b1t.append(t)

                    hts_half = [
                        hp.tile([128, C], f32r, tag=f"h{half}_{hm}", name=f"ht{hm}")
                        for hm in range(n_hm)
                    ]
                    # token-slice outer so the PE can start on slice 0 while
                    # later xT slices are still loading (first chunk only)
                    for t0, tn in TOK_SLICES:
                        for hm in range(n_hm):
                            ps = ps1.tile([128, 384], f32, tag="ps1", name="ps1t")
                            for dk in range(n_dk):
                                nc.tensor.matmul(
                                    ps[:, :tn],
                                    w1t[dk][:, hm * 128 : (hm + 1) * 128],
                                    xt[dk][:, t0 : t0 + tn],
                                    start=(dk == 0),
                                    stop=(dk == n_dk - 1),
                                )
                            nc.vector.tensor_scalar(
                                hts_half[hm][:, t0 : t0 + tn],
                                ps[:, :tn],
                                b1t[hm],
                                0.0,
                                add,
                                mx,
                            )
                    hts.extend(hts_half)

                # W2 slab loads: emitted after the W1 chunk loads so they
                # queue behind them at startup (W1 is needed sooner); in
                # steady state the bufs=1 WAR on last slab's gemm2 gates the
                # start anyway and the load hides under this slab's gemm1.
                w2t = []
                for hk in range(n_hk):
                    t = w2p.tile([128, D], f32r, tag=f"w2_{hk}", name=f"w2t{hk}")
                    h0 = s2 * SLAB2 + hk * 128
                    hwdma(out=t, in_=W2[h0 : h0 + 128, :])
                    w2t.append(t)

                # gemm2: y(+)= hT_slab.T @ W2s
                for tk in range(n_tk):
                    tw = min(128, C - tk * 128)  # last tile is partial
                    for dn in range(n_dn):
                        ps = ps2.tile([128, 512], f32, tag="ps2", name="ps2t")
                        for hk in range(n_hk):
                            nc.tensor.matmul(
                                ps[:tw, :],
                                hts[hk][:, tk * 128 : tk * 128 + tw],
                                w2t[hk][:, dn * 512 : (dn + 1) * 512],
                                start=(hk == 0),
                                stop=(hk == n_hk - 1),
                            )
                        ys = yt[tk][:tw, dn * 512 : (dn + 1) * 512]
                        if s2 == 0:
                            nc.vector.tensor_copy(ys, ps[:tw, :])
                        else:
                            nc.vector.tensor_add(ys, ys, ps[:tw, :])
                        if s2 == n_s2 - 1:
                            hwdma(
                                out=y[tk * 128 : tk * 128 + tw,
                                      dn * 512 : (dn + 1) * 512],
                                in_=ys,
                            )
    nc.compile()
    return nc


def _get_nc():
    if "nc" not in _NC_CACHE:
        _NC_CACHE["nc"] = _build_nc()
    return _NC_CACHE["nc"]


def kernel(x, Wg, bg, W1, b1, W2, b2):
    global LAST_RESULTS
    x = np.asarray(x, dtype=np.float32)
    Wg = np.asarray(Wg, dtype=np.float32)
    bg = np.asarray(bg, dtype=np.float32)
    W1 = np.asarray(W1, dtype=np.float32)
    b1 = np.asarray(b1, dtype=np.float32)
    W2 = np.asarray(W2, dtype=np.float32)
    b2 = np.asarray(b2, dtype=np.float32)

    # --- gate + top-k routing (replicated small gate, on host) ---
    g = x @ Wg + bg  # [N, E]
    order = np.argsort(-g, axis=1, kind="stable")[:, :TOPK]  # [N, 2]
    topv = np.take_along_axis(g, order, axis=1)
    topv = topv - topv.max(axis=1, keepdims=True)
    ex = np.exp(topv)
    sw = ex / ex.sum(axis=1, keepdims=True)  # [N, 2] softmax over selected

    nc = _get_nc()
    in_maps = []
    routing = []
    for e in range(E):
        tok, kk = np.where(order == e)
        cnt = tok.size
        assert cnt <= C, f"expert {e} overflow: {cnt} > {C}"
        xTe = np.zeros((D, C), np.float32)
        xTe[:, :cnt] = x[tok].T
        in_maps.append(
            {
                "xT": xTe,
                "W1": np.ascontiguousarray(W1[e]),
                "W2": np.ascontiguousarray(W2[e]),
                "b1": np.ascontiguousarray(b1[e].reshape(H, 1)),
            }
        )
        routing.append((tok, kk, cnt))

    kwargs = {}
    if TRACE_CORES is not None:
        kwargs["trace_cores"] = TRACE_CORES
    LAST_RESULTS = bass_utils.run_bass_kernel_spmd(
        nc, in_maps, core_ids=list(range(NCORES)), trace=TRACE, **kwargs
    )

    # --- combine: scatter-add gate-weighted expert outputs ---
    out = np.zeros((N_TOK, D), np.float32)
    for e in range(E):
        tok, kk, cnt = routing[e]
        ye = LAST_RESULTS.results[e]["y"][:cnt]
        if np.any(b2[e]):
            ye = ye + b2[e][None, :]
        # token ids are unique within one expert's list, so += is safe
        out[tok] += sw[tok, kk][:, None] * ye
    return out



# revision 3
# speedup vs baseline: 1.0066x; 1.0066x over previous
"""MoE layer (N=4096, D=1024, H=4096, E=8, top-2) on 8 Trainium2 cores.

Strategy: hidden-dim tensor-parallel (replaces expert-parallel).
  - Host computes the small gate, top-2 ids and softmax weights, groups the
    8192 (token, expert) pairs by expert, and replicates the grouped
    activations xT to all 8 cores.
  - Core c holds the hidden slice [c*512, (c+1)*512) of ALL experts' W1/W2
    (SBUF-resident, loaded once) and computes for every pair the partial
    FFN over its slice:
        h = relu(x @ W1[e][:, sl] + b1[e][sl]);  yT_partial = W2[e][sl].T-red
    Every core runs the exact same 8192 pair columns (per-expert group
    sizes baked in at compile time), so there is zero capacity padding and
    perfect load balance regardless of routing skew.
  - Host sums the 8 partial yT outputs, adds b2 and the gate-weighted
    scatter into the [N, D] output.

Device kernel (identical SPMD program on all 8 cores):
  - All matmul operands fp16 (runs at bf16 PE rate, fp32 PSUM accumulation);
    halves SBUF/HBM traffic vs fp32r and has no >=256 free-dim rate rule.
  - Pairs processed in 19 expert-pure chunks of <=512 columns:
      gemm1: h[hm][:, chunk]   = relu(sum_dk W1t.T @ xT)    (ACT drains PSUM)
      gemm2: yT[dc][:, chunk] += sum_hk W2t.T @ h           (DVE drains PSUM)
  - x streams 3 chunks ahead on the sync queue; yT streams out on gpsimd;
    weights load on scalar/gpsimd rings under the first chunks' compute.
"""

import numpy as np

from concourse import bacc
import concourse.mybir as mybir
from concourse.tile import TileContext
import concourse.bass_utils as bass_utils

N_TOK, D, H, E, TOPK = 4096, 1024, 4096, 8, 2
NCORES = 8
PAIRS = N_TOK * TOPK  # 8192 (token, expert) pairs, expert-grouped
HS = H // NCORES      # 512 hidden units per core
DK = D // 128         # 8 contraction tiles for gemm1
HMT = HS // 128       # 4 hidden tiles (gemm1 out / gemm2 contraction)
DCT = D // 128        # 8 output-column tiles for gemm2
CHUNK = 512           # max pair-chunk width (one fp32 PSUM bank)
WARM_MM = 8           # HAM warmup matmuls issued while preamble DMAs land

TRACE = False
TRACE_CORES = None
LAST_RESULTS = None

_NC_CACHE = {}


def _chunks(counts):
    """Expert-pure, balanced pair chunks of width <= CHUNK."""
    out = []
    off = 0
    for e, n in enumerate(counts):
        k = -(-n // CHUNK) if n else 0
        for i in range(k):
            w = n // k + (1 if i < n % k else 0)
            out.append((e, off, w))
            off += w
    return out


def _build_nc(counts):
    f16, f32 = mybir.dt.float16, mybir.dt.float32
    Relu = mybir.ActivationFunctionType.Relu
    nc = bacc.Bacc("TRN2", target_bir_lowering=False)
    xT = nc.dram_tensor("xT", [D, PAIRS], f16, kind="ExternalInput")
    W1 = nc.dram_tensor("W1", [E * D, HS], f16, kind="ExternalInput")
    W2 = nc.dram_tensor("W2", [E * HS, D], f16, kind="ExternalInput")
    b1 = nc.dram_tensor("b1", [128, E * HMT], f32, kind="ExternalInput")
    yT = nc.dram_tensor("yT", [D, PAIRS], f16, kind="ExternalOutput")

    chunks = _chunks(counts)

    with TileContext(nc) as tc:
        with (
            tc.tile_pool(name="w1p", bufs=1) as w1p,
            tc.tile_pool(name="w2p", bufs=1) as w2p,
            tc.tile_pool(name="xp", bufs=3) as xp,
            tc.tile_pool(name="hp", bufs=2) as hp,
            tc.tile_pool(name="yp", bufs=2) as yp,
            tc.tile_pool(name="cp", bufs=1) as cp,
            tc.tile_pool(name="ps1", bufs=3, space="PSUM") as ps1,
            tc.tile_pool(name="ps2", bufs=3, space="PSUM") as ps2,
        ):
            # --- HAM warmup: dummy matmuls keep the PE busy (releasing the
            # clock gate) while the first weight/x DMAs land ---
            warm = cp.tile([128, CHUNK], f16, tag="warm", name="warm")
            nc.vector.memset(warm, 0.0)
            wps = ps2.tile([128, CHUNK], f32, tag="ps2", name="warmps")
            for i in range(WARM_MM):
                nc.tensor.matmul(
                    wps, warm[:, :128], warm, start=(i == 0), stop=(i == WARM_MM - 1)
                )

            # --- b1 + weights: expert 0 first on the scalar ring (needed
            # soonest), remaining experts alternate scalar/gpsimd; both
            # rings drain far ahead of each expert's first chunk ---
            b1t = cp.tile([128, E * HMT], f32, tag="b1", name="b1t")
            nc.gpsimd.dma_start(out=b1t, in_=b1[:, :])
            w1t = [None] * (E * DK)
            w2t = [None] * (E * HMT)
            for e in range(E):
                eng = nc.scalar if e % 2 == 0 else nc.gpsimd
                for dk in range(DK):
                    t = w1p.tile([128, HS], f16, tag=f"w1_{e}_{dk}", name=f"w1t{e}_{dk}")
                    r0 = e * D + dk * 128
                    eng.dma_start(out=t, in_=W1[r0 : r0 + 128, :])
                    w1t[e * DK + dk] = t
                for hk in range(HMT):
                    t = w2p.tile([128, D], f16, tag=f"w2_{e}_{hk}", name=f"w2t{e}_{hk}")
                    r0 = e * HS + hk * 128
                    eng.dma_start(out=t, in_=W2[r0 : r0 + 128, :])
                    w2t[e * HMT + hk] = t

            # --- x pair-chunks: all queued on sync; pool rotation (bufs=3)
            # turns the in-order queue into a 3-deep prefetch pipeline ---
            xtiles = []
            for ci, (e, off, w) in enumerate(chunks):
                xt = []
                for dk in range(DK):
                    t = xp.tile([128, CHUNK], f16, tag=f"x{dk}", name=f"xt{dk}")
                    nc.sync.dma_start(
                        out=t[:, :w], in_=xT[dk * 128 : (dk + 1) * 128, off : off + w]
                    )
                    xt.append(t)
                xtiles.append(xt)

            for ci, (e, off, w) in enumerate(chunks):
                xt = xtiles[ci]
                # gemm1: h[hm] = relu(sum_dk W1.T @ x + b1)
                ht = [
                    hp.tile([128, CHUNK], f16, tag=f"h{hm}", name=f"ht{hm}")
                    for hm in range(HMT)
                ]
                for hm in range(HMT):
                    ps = ps1.tile([128, CHUNK], f32, tag="ps1", name="ps1t")
                    for dk in range(DK):
                        nc.tensor.matmul(
                            ps[:, :w],
                            w1t[e * DK + dk][:, hm * 128 : (hm + 1) * 128],
                            xt[dk][:, :w],
                            start=(dk == 0),
                            stop=(dk == DK - 1),
                        )
                    col = e * HMT + hm
                    nc.scalar.activation(
                        ht[hm][:, :w], ps[:, :w], Relu, bias=b1t[:, col : col + 1]
                    )
                # gemm2: yT[dc] = sum_hk W2.T @ h
                for dc in range(DCT):
                    ps = ps2.tile([128, CHUNK], f32, tag="ps2", name="ps2t")
                    for hk in range(HMT):
                        nc.tensor.matmul(
                            ps[:, :w],
                            w2t[e * HMT + hk][:, dc * 128 : (dc + 1) * 128],
                            ht[hk][:, :w],
                            start=(hk == 0),
                            stop=(hk == HMT - 1),
                        )
                    yt = yp.tile([128, CHUNK], f16, tag=f"y{dc}", name=f"yt{dc}")
                    nc.vector.tensor_copy(yt[:, :w], ps[:, :w])
                    nc.gpsimd.dma_start(
                        out=yT[dc * 128 : (dc + 1) * 128, off : off + w], in_=yt[:, :w]
                    )
    nc.compile()
    return nc


def _get_nc(counts):
    if counts not in _NC_CACHE:
        _NC_CACHE[counts] = _build_nc(counts)
    return _NC_CACHE[counts]


def kernel(x, Wg, bg, W1, b1, W2, b2):
    global LAST_RESULTS
    x = np.asarray(x, dtype=np.float32)
    Wg = np.asarray(Wg, dtype=np.float32)
    bg = np.asarray(bg, dtype=np.float32)
    W1 = np.asarray(W1, dtype=np.float32)
    b1 = np.asarray(b1, dtype=np.float32)
    W2 = np.asarray(W2, dtype=np.float32)
    b2 = np.asarray(b2, dtype=np.float32)

    # --- gate + top-k routing (replicated small gate, on host) ---
    g = x @ Wg + bg  # [N, E]
    order = np.argsort(-g, axis=1, kind="stable")[:, :TOPK]  # [N, 2]
    topv = np.take_along_axis(g, order, axis=1)
    topv = topv - topv.max(axis=1, keepdims=True)
    ex = np.exp(topv)
    sw = ex / ex.sum(axis=1, keepdims=True)  # [N, 2] softmax over selected

    counts = tuple(int((order == e).sum()) for e in range(E))
    nc = _get_nc(counts)

    # --- dispatch: expert-grouped pair order, replicated to all cores ---
    pos = np.empty((N_TOK, TOPK), np.int64)  # (token, k) -> pair column
    offs = []
    toks = []
    off = 0
    for e in range(E):
        tok, kk = np.where(order == e)
        pos[tok, kk] = off + np.arange(tok.size)
        offs.append(off)
        toks.append(tok)
        off += tok.size
    tok_all = np.concatenate(toks)
    xTP = np.ascontiguousarray(x[tok_all].T).astype(np.float16)  # [D, PAIRS]

    in_maps = []
    for c in range(NCORES):
        sl = slice(c * HS, (c + 1) * HS)
        W1s = np.ascontiguousarray(W1[:, :, sl]).astype(np.float16).reshape(E * D, HS)
        W2s = np.ascontiguousarray(W2[:, sl, :]).astype(np.float16).reshape(E * HS, D)
        b1s = np.ascontiguousarray(
            b1[:, sl].reshape(E, HMT, 128).transpose(2, 0, 1).reshape(128, E * HMT)
        )
        in_maps.append({"xT": xTP, "W1": W1s, "W2": W2s, "b1": b1s})

    kwargs = {}
    if TRACE_CORES is not None:
        kwargs["trace_cores"] = TRACE_CORES
    LAST_RESULTS = bass_utils.run_bass_kernel_spmd(
        nc, in_maps, core_ids=list(range(NCORES)), trace=TRACE, **kwargs
    )

    # --- combine: sum partials over cores, add b2, gate-weighted scatter ---
    Ysum = np.zeros((D, PAIRS), np.float32)
    for r in LAST_RESULTS.results:
        Ysum += r["yT"].astype(np.float32)
    Y = np.ascontiguousarray(Ysum.T)  # [PAIRS, D]
    for e in range(E):
        if np.any(b2[e]):
            Y[offs[e] : offs[e] + counts[e]] += b2[e][None, :]
    out = sw[:, 0, None] * Y[pos[:, 0]] + sw[:, 1, None] * Y[pos[:, 1]]
    return out.astype(np.float32)


# revision 4
# speedup vs baseline: 1.1602x; 1.1526x over previous
"""MoE layer (N=4096, D=1024, H=4096, E=8, top-2) on 8 Trainium2 cores.

Strategy: hidden-dim tensor-parallel (replaces expert-parallel).
  - Host computes the small gate, top-2 ids and softmax weights, groups the
    8192 (token, expert) pairs by expert, and replicates the grouped
    activations xT to all 8 cores.
  - Core c holds the hidden slice [c*512, (c+1)*512) of ALL experts' W1/W2
    (SBUF-resident, loaded once) and computes for every pair the partial
    FFN over its slice:
        h = relu(x @ W1[e][:, sl] + b1[e][sl]);  yT_partial = h.T-red @ W2
    Every core runs the exact same 8192 pair columns (per-expert group
    sizes baked in at compile time), so there is zero capacity padding and
    perfect load balance regardless of routing skew.
  - Host sums the 8 partial yT outputs, adds b2 and the gate-weighted
    scatter into the [N, D] output.

Device kernel (identical SPMD program on all 8 cores):
  - All matmul operands fp16 (bf16 PE rate, fp32 PSUM accumulation);
    halves SBUF/HBM traffic vs fp32r, no >=256 free-dim rate rule, and the
    lower data-movement power avoids the P0 PE down-clock seen with f32r.
  - Pairs processed in expert-pure chunks of <=512 columns:
      gemm1: h[hm][:, chunk]  = relu(sum_dk W1t.T @ xT)    (ACT drains PSUM)
      gemm2: yT[dc][:, chunk] = sum_hk W2t.T @ h           (DVE drains PSUM)
  - DMA rings are packet-rate limited (~100 partition-lines/us), so each
    ring gets ~16.8MB with fat lines: x streams on sync in 2-chunk groups
    (~2KB lines); weights load on scalar as ONE packed [128, 4096]
    descriptor per expert per matrix (8KB lines, all done in ~50us);
    yT streams out per chunk on gpsimd.  The last chunk is small and its
    output dual-rings over scalar+gpsimd to shrink the drain tail.
"""

import numpy as np

from concourse import bacc
import concourse.mybir as mybir
from concourse.tile import TileContext
import concourse.bass_utils as bass_utils

N_TOK, D, H, E, TOPK = 4096, 1024, 4096, 8, 2
NCORES = 8
PAIRS = N_TOK * TOPK  # 8192 (token, expert) pairs, expert-grouped
HS = H // NCORES      # 512 hidden units per core
DK = D // 128         # 8 contraction tiles for gemm1
HMT = HS // 128       # 4 hidden tiles (gemm1 out / gemm2 contraction)
DCT = D // 128        # 8 output-column tiles for gemm2
CHUNK = 512           # max pair-chunk width (one fp32 PSUM bank)
WARM_MM = 14          # HAM warmup matmuls issued while preamble DMAs land
TAIL_W = 192          # width of the final drain-friendly chunk

TRACE = False
TRACE_CORES = None
LAST_RESULTS = None

_NC_CACHE = {}


def _chunks(counts):
    """Expert-pure, balanced pair chunks of width <= CHUNK.

    The very last chunk is split small (TAIL_W) so the final yT drain after
    the last matmul is short.
    """
    out = []
    off = 0
    for e, n in enumerate(counts):
        k = -(-n // CHUNK) if n else 0
        for i in range(k):
            w = n // k + (1 if i < n % k else 0)
            out.append([e, off, w])
            off += w
    if out and out[-1][2] > TAIL_W + 64:
        e, off, w = out[-1]
        out[-1] = [e, off, w - TAIL_W]
        out.append([e, off + w - TAIL_W, TAIL_W])
    return [tuple(c) for c in out]


def _xgroups(chunks):
    """Group consecutive chunks (2 per group, <= 2*CHUNK wide) for fatter
    x DMA lines.  The first group is a single chunk so compute can start
    as early as possible."""
    groups = []
    i = 0
    first = True
    while i < len(chunks):
        if not first and i + 1 < len(chunks):
            groups.append((i, 2))
            i += 2
        else:
            groups.append((i, 1))
            i += 1
        first = False
    return groups


def _build_nc(counts):
    f16, f32 = mybir.dt.float16, mybir.dt.float32
    Relu = mybir.ActivationFunctionType.Relu
    nc = bacc.Bacc("TRN2", target_bir_lowering=False)
    xT = nc.dram_tensor("xT", [D, PAIRS], f16, kind="ExternalInput")
    W1 = nc.dram_tensor("W1", [E * 128, DK * HS], f16, kind="ExternalInput")
    W2 = nc.dram_tensor("W2", [E * 128, HMT * D], f16, kind="ExternalInput")
    b1 = nc.dram_tensor("b1", [128, E * HMT], f32, kind="ExternalInput")
    yT = nc.dram_tensor("yT", [D, PAIRS], f16, kind="ExternalOutput")

    chunks = _chunks(counts)
    groups = _xgroups(chunks)
    n_chunks = len(chunks)

    with TileContext(nc) as tc:
        with (
            tc.tile_pool(name="w1p", bufs=1) as w1p,
            tc.tile_pool(name="w2p", bufs=1) as w2p,
            tc.tile_pool(name="xp", bufs=2) as xp,
            tc.tile_pool(name="hp", bufs=2) as hp,
            tc.tile_pool(name="yp", bufs=2) as yp,
            tc.tile_pool(name="cp", bufs=1) as cp,
            tc.tile_pool(name="ps1", bufs=3, space="PSUM") as ps1,
            tc.tile_pool(name="ps2", bufs=3, space="PSUM") as ps2,
        ):
            # --- HAM warmup: dummy matmuls keep the PE busy (releasing the
            # clock gate) while the first weight/x DMAs land ---
            warm = cp.tile([128, CHUNK], f16, tag="warm", name="warm")
            nc.vector.memset(warm, 0.0)
            wps = ps2.tile([128, CHUNK], f32, tag="ps2", name="warmps")
            for i in range(WARM_MM):
                nc.tensor.matmul(
                    wps, warm[:, :128], warm, start=(i == 0), stop=(i == WARM_MM - 1)
                )

            # --- weights: one packed descriptor per expert per matrix on
            # the scalar ring (8KB lines); the whole stream lands in ~50us
            # while only expert 0 is needed in the first ~25us ---
            b1t = cp.tile([128, E * HMT], f32, tag="b1", name="b1t")
            nc.gpsimd.dma_start(out=b1t, in_=b1[:, :])
            w1t = []
            w2t = []
            for e in range(E):
                t1 = w1p.tile([128, DK * HS], f16, tag=f"w1_{e}", name=f"w1t{e}")
                nc.scalar.dma_start(out=t1, in_=W1[e * 128 : (e + 1) * 128, :])
                w1t.append(t1)
                t2 = w2p.tile([128, HMT * D], f16, tag=f"w2_{e}", name=f"w2t{e}")
                nc.scalar.dma_start(out=t2, in_=W2[e * 128 : (e + 1) * 128, :])
                w2t.append(t2)

            # --- x pair-chunks: grouped descriptors on sync; pool rotation
            # (bufs=2 of 2-chunk groups) gives a ~4-chunk prefetch window ---
            xtiles = [None] * n_chunks
            for gi, (c0, ng) in enumerate(groups):
                off = chunks[c0][1]
                gw = sum(chunks[c0 + j][2] for j in range(ng))
                gt = []
                for dk in range(DK):
                    t = xp.tile([128, 2 * CHUNK], f16, tag=f"x{dk}", name=f"xt{dk}")
                    nc.sync.dma_start(
                        out=t[:, :gw], in_=xT[dk * 128 : (dk + 1) * 128, off : off + gw]
                    )
                    gt.append(t)
                sub = 0
                for j in range(ng):
                    w = chunks[c0 + j][2]
                    xtiles[c0 + j] = [gt[dk][:, sub : sub + w] for dk in range(DK)]
                    sub += w

            for ci, (e, off, w) in enumerate(chunks):
                xt = xtiles[ci]
                # gemm1: h[hm] = relu(sum_dk W1.T @ x + b1)
                ht = [
                    hp.tile([128, CHUNK], f16, tag=f"h{hm}", name=f"ht{hm}")
                    for hm in range(HMT)
                ]
                for hm in range(HMT):
                    ps = ps1.tile([128, CHUNK], f32, tag="ps1", name="ps1t")
                    for dk in range(DK):
                        nc.tensor.matmul(
                            ps[:, :w],
                            w1t[e][:, dk * HS + hm * 128 : dk * HS + (hm + 1) * 128],
                            xt[dk],
                            start=(dk == 0),
                            stop=(dk == DK - 1),
                        )
                    col = e * HMT + hm
                    nc.scalar.activation(
                        ht[hm][:, :w], ps[:, :w], Relu, bias=b1t[:, col : col + 1]
                    )
                # gemm2: yT[dc] = sum_hk W2.T @ h
                for dc in range(DCT):
                    ps = ps2.tile([128, CHUNK], f32, tag="ps2", name="ps2t")
                    for hk in range(HMT):
                        nc.tensor.matmul(
                            ps[:, :w],
                            w2t[e][:, hk * D + dc * 128 : hk * D + (dc + 1) * 128],
                            ht[hk][:, :w],
                            start=(hk == 0),
                            stop=(hk == HMT - 1),
                        )
                    yt = yp.tile([128, CHUNK], f16, tag=f"y{dc}", name=f"yt{dc}")
                    nc.vector.tensor_copy(yt[:, :w], ps[:, :w])
                    # last two chunks: split the output drain across two
                    # rings so the post-compute tail is short
                    if ci >= n_chunks - 2 and dc % 2 == 1:
                        oeng = nc.scalar
                    else:
                        oeng = nc.gpsimd
                    oeng.dma_start(
                        out=yT[dc * 128 : (dc + 1) * 128, off : off + w], in_=yt[:, :w]
                    )
    nc.compile()
    return nc


def _get_nc(counts):
    if counts not in _NC_CACHE:
        _NC_CACHE[counts] = _build_nc(counts)
    return _NC_CACHE[counts]


def kernel(x, Wg, bg, W1, b1, W2, b2):
    global LAST_RESULTS
    x = np.asarray(x, dtype=np.float32)
    Wg = np.asarray(Wg, dtype=np.float32)
    bg = np.asarray(bg, dtype=np.float32)
    W1 = np.asarray(W1, dtype=np.float32)
    b1 = np.asarray(b1, dtype=np.float32)
    W2 = np.asarray(W2, dtype=np.float32)
    b2 = np.asarray(b2, dtype=np.float32)

    # --- gate + top-k routing (replicated small gate, on host) ---
    g = x @ Wg + bg  # [N, E]
    order = np.argsort(-g, axis=1, kind="stable")[:, :TOPK]  # [N, 2]
    topv = np.take_along_axis(g, order, axis=1)
    topv = topv - topv.max(axis=1, keepdims=True)
    ex = np.exp(topv)
    sw = ex / ex.sum(axis=1, keepdims=True)  # [N, 2] softmax over selected

    counts = tuple(int((order == e).sum()) for e in range(E))
    nc = _get_nc(counts)

    # --- dispatch: expert-grouped pair order, replicated to all cores ---
    pos = np.empty((N_TOK, TOPK), np.int64)  # (token, k) -> pair column
    offs = []
    toks = []
    off = 0
    for e in range(E):
        tok, kk = np.where(order == e)
        pos[tok, kk] = off + np.arange(tok.size)
        offs.append(off)
        toks.append(tok)
        off += tok.size
    tok_all = np.concatenate(toks)
    xTP = np.ascontiguousarray(x[tok_all].T).astype(np.float16)  # [D, PAIRS]

    in_maps = []
    for c in range(NCORES):
        sl = slice(c * HS, (c + 1) * HS)
        # pack each expert's weight slice as one [128, DK*HS] / [128, HMT*D]
        # row-block so it loads as a single fat-lined DMA descriptor
        W1s = np.ascontiguousarray(
            W1[:, :, sl]
            .reshape(E, DK, 128, HS)
            .transpose(0, 2, 1, 3)
            .reshape(E * 128, DK * HS)
        ).astype(np.float16)
        W2s = np.ascontiguousarray(
            W2[:, sl, :]
            .reshape(E, HMT, 128, D)
            .transpose(0, 2, 1, 3)
            .reshape(E * 128, HMT * D)
        ).astype(np.float16)
        b1s = np.ascontiguousarray(
            b1[:, sl].reshape(E, HMT, 128).transpose(2, 0, 1).reshape(128, E * HMT)
        )
        in_maps.append({"xT": xTP, "W1": W1s, "W2": W2s, "b1": b1s})

    kwargs = {}
    if TRACE_CORES is not None:
        kwargs["trace_cores"] = TRACE_CORES
    LAST_RESULTS = bass_utils.run_bass_kernel_spmd(
        nc, in_maps, core_ids=list(range(NCORES)), trace=TRACE, **kwargs
    )

    # --- combine: sum partials over cores, add b2, gate-weighted scatter ---
    Ysum = np.zeros((D, PAIRS), np.float32)
    for r in LAST_RESULTS.results:
        Ysum += r["yT"].astype(np.float32)
    Y = np.ascontiguousarray(Ysum.T)  # [PAIRS, D]
    for e in range(E):
        if np.any(b2[e]):
            Y[offs[e] : offs[e] + counts[e]] += b2[e][None, :]
    out = sw[:, 0, None] * Y[pos[:, 0]] + sw[:, 1, None] * Y[pos[:, 1]]
    return out.astype(np.float32)
